# revision 3
# baseline (speedup 1.0000x reference)
"""FP8 block-wise dequant linear: out[b,s,o] = sum_i x[b,s,i] * (w[o,i]*scale[o//128,i//128]).

Sharding: 4-way over seq x 2-way over out_features across 8 NeuronCores.
Per core: x shard [512 seq, 4096 in] (bf16, host-precast), w shard
[2048 out, 4096 in] fp8, out [512, 2048] (bf16 on device, f32 on host).

Device pipeline per core (stationary-x, out[s, o] orientation):
  - Persistent SBUF tiles (xb, wq, sc) are allocated once, outside the
    timing loop; each loop body re-DMAs/re-dequantizes into them, so
    Tile's subregion-granular WAR deps let body k+1's loads overlap
    body k's matmuls (no slot-recycle serialization).
  - x is staged sb-major (4 x 1 MiB pieces): piece sb is last read 1/4
    into a body, leaving a ~85 us cross-body reload window.
  - w fp8 streams through small staging tiles in kb-groups [1,3,4x7]
    (first group small so the post-back-edge refill beats the ~3.4 us
    HAM window); DVE dequantizes each kb slab [p, 16ob x 128] to bf16
    in one tensor_tensor with free-dim-broadcast scales. The resident
    bf16 wq slab (kb) is freed by the last sb phase ~28 us before body
    k+1 needs it, and the dequant itself needs only ~0.5 us.
  - TensorE: sb-outer / kb-inner / oc-inner: each stationary x-block
    load feeds 4 N=512 bf16 matmuls (the LDW cost, ~80 cycles visible,
    amortizes 4x); 32 kb accumulate in 4 PSUM banks, ping-ponged with
    the next sb's 4. ScalarE evacuates to bf16; DMA out.
  - Timing loop: For_i over UNROLL=8 bodies + hint_engines=(PE,) so the
    ~2 us back-edge barrier + PE IRAM refetch amortize to <0.5 us/body.

Roofline: 512 N=512 bf16 matmuls/core; measured pair cost ~222 ns
(512+~80/4 cycles @2.4 GHz + 2.5 ns NX) -> ~113.9 us PE floor. Total
DMA 14.25 MiB/body (~42 us at 358 GB/s) fully prefetched; kernel stays
PE-bound down to ~130 GB/s effective DMA bandwidth.
"""

import numpy as np
import ml_dtypes

import concourse.bacc as bacc
import concourse.mybir as mybir
from concourse.tile import TileContext
from concourse.bass_utils import run_bass_kernel_spmd

SEQ, DIN, DOUT = 2048, 4096, 4096
N_CORES = 8
SEQ_SHARDS, OUT_SHARDS = 4, 2
SEQ_SH, OUT_SH = SEQ // SEQ_SHARDS, DOUT // OUT_SHARDS  # 512, 2048
P = 128
NKB = DIN // P            # 32 contraction blocks
NOB = OUT_SH // P         # 16 out blocks per core
NMM = 512                 # matmul moving free dim (PSUM one-bank limit)
NOC = OUT_SH // NMM       # 4 out chunks per core
NSB = SEQ_SH // P         # 4 seq blocks per core
SBW = NKB * P             # 4096 x cols per sb piece

WGROUPS = [[0], [1, 2, 3]] + [list(range(k, k + 4)) for k in range(4, NKB, 4)]
UNROLL = 8                # bodies per For_i iteration


def emit_body(nc, persist, pools, io):
    dt = mybir.dt
    wf_pool, ob_pool, ps_pool = pools
    xt, wt, sc, out = io
    xb_all, wq_all, sc_sb = persist

    nc.sync.dma_start(sc_sb[:], sc[:])
    for sb in range(NSB):
        nc.gpsimd.dma_start(xb_all[:, sb * SBW:(sb + 1) * SBW],
                            xt[:, sb * SBW:(sb + 1) * SBW])

    for g in WGROUPS:
        wf = wf_pool.tile([P, 4 * OUT_SH], dt.float8e4, tag="wf")
        nc.gpsimd.dma_start(wf[:, :len(g) * OUT_SH],
                            wt[:, g[0] * OUT_SH:(g[-1] + 1) * OUT_SH])
        for j, kb in enumerate(g):
            s_b = (sc_sb[:, kb * NOB:(kb + 1) * NOB]
                   .unsqueeze(2).broadcast_to([P, NOB, P]))
            nc.vector.tensor_mul(
                wq_all[:, kb * OUT_SH:(kb + 1) * OUT_SH]
                .rearrange("p (b i) -> p b i", b=NOB),
                wf[:, j * OUT_SH:(j + 1) * OUT_SH]
                .rearrange("p (b i) -> p b i", b=NOB),
                s_b,
            )

    for sb in range(NSB):
        pss = []
        for oc in range(NOC):
            ps = ps_pool.tile([P, NMM], dt.float32, tag="ps")
            pss.append(ps)
        for kb in range(NKB):
            lhs = xb_all[:, sb * SBW + kb * P:sb * SBW + (kb + 1) * P]
            for oc in range(NOC):
                nc.tensor.matmul(
                    pss[oc][:],
                    lhs,
                    wq_all[:, kb * OUT_SH + oc * NMM:
                              kb * OUT_SH + (oc + 1) * NMM],
                    start=(kb == 0),
                    stop=(kb == NKB - 1),
                )
        for half in range(0, NOC, 2):
            ob = ob_pool.tile([P, 2 * NMM], dt.bfloat16, tag="ob")
            for j in range(2):
                nc.scalar.copy(ob[:, j * NMM:(j + 1) * NMM], pss[half + j][:])
            nc.gpsimd.dma_start(
                out[:, sb * OUT_SH + half * NMM:
                       sb * OUT_SH + (half + 2) * NMM],
                ob[:])


def build_nc(iters=1, loop=None, **kw):
    nc = bacc.Bacc(None, target_bir_lowering=False)
    xt = nc.dram_tensor("xt", [P, NSB * SBW], mybir.dt.bfloat16, kind="ExternalInput")
    wt = nc.dram_tensor("wt", [P, NKB * OUT_SH], mybir.dt.float8e4, kind="ExternalInput")
    sc = nc.dram_tensor("sc", [P, NKB * NOB], mybir.dt.float32, kind="ExternalInput")
    out = nc.dram_tensor("out", [P, NSB * OUT_SH], mybir.dt.bfloat16,
                         kind="ExternalOutput")
    io = (xt, wt, sc, out)
    dt = mybir.dt

    with TileContext(nc) as tc:
        with (
            tc.tile_pool(name="persist", bufs=1) as persist_pool,
            tc.tile_pool(name="wf", bufs=2) as wf_pool,
            tc.tile_pool(name="ob", bufs=3) as ob_pool,
            tc.tile_pool(name="ps", bufs=8, space="PSUM") as ps_pool,
        ):
            xb_all = persist_pool.tile([P, NSB * SBW], dt.bfloat16, tag="xb")
            wq_all = persist_pool.tile([P, NKB * OUT_SH], dt.bfloat16, tag="wq")
            sc_sb = persist_pool.tile([P, NKB * NOB], dt.float32, tag="sc")
            persist = (xb_all, wq_all, sc_sb)
            pools = (wf_pool, ob_pool, ps_pool)
            if loop is not None:
                assert loop % UNROLL == 0, (loop, UNROLL)
                with tc.For_i(0, loop // UNROLL, 1,
                              hint_engines=(mybir.EngineType.PE,)):
                    for _ in range(UNROLL):
                        emit_body(nc, persist, pools, io)
            else:
                for _ in range(iters):
                    emit_body(nc, persist, pools, io)
    nc.compile()
    return nc


def shard_inputs(x, weight, weight_scale_inv):
    """Host staging, partition-major per core:
       xt[p, sb*SBW+kb*P+f] = x[0][si*SEQ_SH+sb*128+f, kb*128+p]  (bf16)
       wt[p, kb*OUT_SH+o]   = weight[oi*OUT_SH+o, kb*128+p]       (fp8)
       sc[p, kb*NOB+ob]     = weight_scale_inv[oi*NOB+ob, kb]     (f32)"""
    x = np.asarray(x)
    weight = np.asarray(weight)
    scale = np.asarray(weight_scale_inv, dtype=np.float32)
    w8 = weight.view(np.uint8)

    in_maps = []
    x_dev = {}
    w_dev = {}
    for c in range(N_CORES):
        si, oi = c % SEQ_SHARDS, c // SEQ_SHARDS
        if si not in x_dev:
            xs = np.asarray(x[0][si * SEQ_SH:(si + 1) * SEQ_SH, :],
                            dtype=np.float32).astype(ml_dtypes.bfloat16)
            # [(sb,f), (kb,p)] -> [p, sb, kb, f]
            x_dev[si] = np.ascontiguousarray(
                xs.reshape(NSB, P, NKB, P).transpose(3, 0, 2, 1)
            ).reshape(P, NSB * SBW)
        if oi not in w_dev:
            ws = w8[oi * OUT_SH:(oi + 1) * OUT_SH, :]
            w_dev[oi] = np.ascontiguousarray(
                ws.T.reshape(NKB, P, OUT_SH).transpose(1, 0, 2)
            ).reshape(P, NKB * OUT_SH).view(ml_dtypes.float8_e4m3)
        sc_core = scale.T[:, oi * NOB:(oi + 1) * NOB]        # [NKB, NOB]
        sc_st = np.ascontiguousarray(
            np.broadcast_to(sc_core.reshape(1, NKB * NOB), (P, NKB * NOB)))
        in_maps.append({"xt": x_dev[si], "wt": w_dev[oi], "sc": sc_st})
    return in_maps


def unshard_output(results):
    out = np.empty((1, SEQ, DOUT), dtype=np.float32)
    for c in range(N_CORES):
        si, oi = c % SEQ_SHARDS, c // SEQ_SHARDS
        o = np.asarray(results[c]["out"], dtype=np.float32)
        # out[p, sb*OUT_SH+o] = y[sb*128+p, o]
        y = o.reshape(P, NSB, OUT_SH).transpose(1, 0, 2).reshape(SEQ_SH, OUT_SH)
        out[0, si * SEQ_SH:(si + 1) * SEQ_SH,
            oi * OUT_SH:(oi + 1) * OUT_SH] = y
    return out


_NC_CACHE = {}


def _run_spmd(nc, in_maps, tries=3):
    """The axon-tunneled device occasionally faults with
    NRT_EXEC_UNIT_UNRECOVERABLE, which poisons the whole PJRT client —
    reset jax backends before retrying."""
    import time as _time
    last = None
    for t in range(tries):
        try:
            return run_bass_kernel_spmd(nc, in_maps, core_ids=list(range(N_CORES)))
        except Exception as e:  # noqa: BLE001
            last = e
            _time.sleep(2.0)
            try:
                import jax as _jax
                _jax.clear_backends()
            except Exception:  # noqa: BLE001
                pass
    raise last


def kernel(x, weight, weight_scale_inv):
    if "nc" not in _NC_CACHE:
        _NC_CACHE["nc"] = build_nc()
    nc = _NC_CACHE["nc"]
    in_maps = shard_inputs(x, weight, weight_scale_inv)
    res = _run_spmd(nc, in_maps)
    return unshard_output(res.results)
